# revision 20
# baseline (speedup 1.0000x reference)
"""Masked dot-product attention, sharded over 8 NeuronCores.

Problem: B=2, H=8, S=4096, D=64; mask [1,1,S,S] broadcast over (b,h).
  scores = (Q @ K^T) / sqrt(d_k); masked_fill(~mask, -1e9); softmax;
  weights = min(mask, weights)  (== zero masked weights); out = weights @ V.

Strategy (per core: 2 of the 16 (b,h) pairs, mask replicated):
  Everything is computed in a transposed layout so no on-chip transposes
  are needed:
    s^T[k,q] = sum_d K^T[d,k] Q^T[d,q]      (PE, bf16, k-chunks of 128)
    s += 832*maskT                          (DVE scalar_tensor_tensor, in PSUM)
    w^T = exp(s/8 - 104)                    (ACT; masked lanes -> ~0)
    o^T[d,q] += V_aug^T chunks @ w^T        (PE bf16, accumulate over k; V_aug
                                             has an appended ones column so row
                                             D is the softmax denominator)
  Host side: pre-transposes Q,K (bf16) and the mask (once), appends the ones
  column to V (bf16), and does the final divide + transpose of the [D+1, S]
  fp32 output.
"""

import os
from contextlib import ExitStack

import numpy as np

B, H, S, D = 2, 8, 4096, 64
N_CORES = 8
BH_PER_CORE = (B * H) // N_CORES  # 2
KC = 128    # k-chunk size (partition dim of s^T tiles)
QB = 512    # q columns per matmul (PSUM bank limit for fp32 out)
FB = 1024   # fused free-dim for DVE/ACT elementwise ops

_BUILT = {}
LAST_RESULTS = None  # BassKernelResults of the most recent run (for test.py)


def _build(nbh, s_len, d_head, scale):
    """Build the Bass program (same SPMD program for every core)."""
    import concourse.bass as bass
    import concourse.mybir as mybir
    import concourse.tile as tile
    from concourse import bacc

    f32 = mybir.dt.float32
    bf16 = mybir.dt.bfloat16
    u8 = mybir.dt.uint8
    fb_size = min(FB, s_len)
    n_kc = s_len // KC
    n_fb = s_len // fb_size
    per_fb = fb_size // QB  # matmuls per fused elementwise block
    dp = d_head + 1

    nc = bacc.Bacc()
    qT = nc.dram_tensor("qT", [nbh, d_head, s_len], bf16, kind="ExternalInput")
    kT = nc.dram_tensor("kT", [nbh, d_head, s_len], bf16, kind="ExternalInput")
    vA = nc.dram_tensor("vA", [nbh, s_len, dp], bf16, kind="ExternalInput")
    mT = nc.dram_tensor("mT", [s_len, s_len], u8, kind="ExternalInput")
    outT = nc.dram_tensor("outT", [nbh, dp, s_len], f32, kind="ExternalOutput")

    with ExitStack() as ctx:
        tc = ctx.enter_context(tile.TileContext(nc))
        consts = ctx.enter_context(tc.tile_pool(name="consts", bufs=1))
        masks = ctx.enter_context(tc.tile_pool(name="masks", bufs=1))
        qk = ctx.enter_context(tc.tile_pool(name="qk", bufs=2))
        vpool = ctx.enter_context(tc.tile_pool(name="vpool", bufs=2))
        wpool = ctx.enter_context(tc.tile_pool(name="wpool", bufs=6))
        opool = ctx.enter_context(tc.tile_pool(name="opool", bufs=3))
        psum_s = ctx.enter_context(tc.tile_pool(name="psum_s", bufs=3, space="PSUM"))
        psum_o = ctx.enter_context(tc.tile_pool(name="psum_o", bufs=2, space="PSUM"))

        bias_sb = consts.tile([KC, 1], f32)
        nc.vector.memset(bias_sb, -104.0)

        # PE warm-up: ~7us of dummy matmuls with no DMA dependency, issued
        # while the first input DMAs are in flight. Trips the HAM clock
        # un-gate (~3.4us sustained activity) so real matmuls start at 2.4GHz.
        warm_in = consts.tile([KC, QB], bf16)
        nc.vector.memset(warm_in, 0.0)
        for wi in range(40):
            warm_ps = psum_s.tile([KC, QB], f32, tag="s_ps")
            nc.tensor.matmul(
                out=warm_ps,
                lhsT=warm_in[:, :KC],
                rhs=warm_in,
                start=True,
                stop=True,
            )

        # Issue the small q/k/v input DMAs for ALL bh pairs first so the PE's
        # first real matmuls aren't stuck behind the 16 MB mask load.
        in_tiles = []
        n_strip = 8
        strip = s_len // n_strip
        for b in range(nbh):
            qt_sb = qk.tile([d_head, s_len], bf16, tag="qt")
            kt_sb = qk.tile([d_head, s_len], bf16, tag="kt")
            va_sb = vpool.tile([KC, n_kc, dp], bf16, tag="va")
            va_r = vA[b].rearrange("(c p) d -> p c d", p=KC)
            # Striped loads spread across DMA queues so the first strips land
            # in a few microseconds instead of one ~20us single-queue DMA.
            for st in range(n_strip):
                sl = slice(st * strip, (st + 1) * strip)
                nc.sync.dma_start(out=kt_sb[:, sl], in_=kT[b][:, sl])
                nc.sync.dma_start(out=qt_sb[:, sl], in_=qT[b][:, sl])
                cl = slice(st * (n_kc // n_strip), (st + 1) * (n_kc // n_strip))
                nc.sync.dma_start(out=va_sb[:, cl], in_=va_r[:, cl])
            in_tiles.append((qt_sb, kt_sb, va_sb))

        # Resident mask^T, one SBUF tile per k-chunk so compute only waits on
        # the chunk it needs (not the whole 16 MB load).
        m_sb = masks.tile([KC, n_kc, s_len], u8)
        mT_r = mT.rearrange("(c p) q -> p c q", p=KC)
        for c in range(n_kc):
            nc.sync.dma_start(out=m_sb[:, c, :], in_=mT_r[:, c, :])

        for b in range(nbh):
            qt_sb, kt_sb, va_sb = in_tiles[b]

            for fb in range(n_fb):
                fs = slice(fb * fb_size, (fb + 1) * fb_size)
                o_ps = []
                for j in range(per_fb):
                    o_tile = psum_o.tile([dp, QB], f32, tag="o_ps")
                    o_ps.append(o_tile)

                for c in range(n_kc):
                    s_ps = psum_s.tile([KC, fb_size], f32, tag="s_ps")
                    for j in range(per_fb):
                        q0 = fb * fb_size + j * QB
                        nc.tensor.matmul(
                            out=s_ps[:, j * QB:(j + 1) * QB],
                            lhsT=kt_sb[:, c * KC:(c + 1) * KC],
                            rhs=qt_sb[:, q0:q0 + QB],
                            start=True,
                            stop=True,
                        )
                    # s += 832*mask  (in place in PSUM, masked lanes jump +832)
                    nc.vector.scalar_tensor_tensor(
                        out=s_ps,
                        in0=m_sb[:, c, fs],
                        scalar=832.0,
                        in1=s_ps,
                        op0=mybir.AluOpType.mult,
                        op1=mybir.AluOpType.add,
                    )
                    # w = exp(s*scale - 104): unmasked -> exp(score/8), masked -> ~0
                    w_sb = wpool.tile([KC, fb_size], bf16, tag="w")
                    nc.scalar.activation(
                        out=w_sb,
                        in_=s_ps,
                        func=mybir.ActivationFunctionType.Exp,
                        bias=bias_sb[:],
                        scale=scale,
                    )
                    for j in range(per_fb):
                        nc.tensor.matmul(
                            out=o_ps[j],
                            lhsT=va_sb[:, c, :],
                            rhs=w_sb[:, j * QB:(j + 1) * QB],
                            start=(c == 0),
                            stop=(c == n_kc - 1),
                        )
                for j in range(per_fb):
                    q0 = fb * fb_size + j * QB
                    o_sb = opool.tile([dp, QB], f32, tag="o")
                    nc.scalar.copy(o_sb, o_ps[j])
                    nc.sync.dma_start(out=outT[b, :, q0:q0 + QB], in_=o_sb)
    return nc


def _get_nc(nbh, s_len, d_head, scale):
    key = (nbh, s_len, d_head, scale)
    if key not in _BUILT:
        nc = _build(*key)
        nc.finalize()
        _BUILT[key] = nc
    return _BUILT[key]


def kernel(queries, keys, values, d_k, mask):
    global LAST_RESULTS
    import ml_dtypes
    from concourse.bass_utils import run_bass_kernel_spmd

    bf16 = ml_dtypes.bfloat16
    queries = np.asarray(queries, dtype=np.float32)
    keys = np.asarray(keys, dtype=np.float32)
    values = np.asarray(values, dtype=np.float32)
    scale = 1.0 / float(np.sqrt(np.float32(np.asarray(d_k))))

    b, h, s_len, d_head = queries.shape
    nbh = (b * h) // N_CORES
    dp = d_head + 1

    q_f = queries.reshape(b * h, s_len, d_head)
    k_f = keys.reshape(b * h, s_len, d_head)
    v_f = values.reshape(b * h, s_len, d_head)

    # mask -> uint8, transposed (shared across all cores)
    m2 = np.asarray(mask).reshape(s_len, s_len)
    mT = np.ascontiguousarray(m2.astype(np.uint8).T)

    v_aug = np.empty((b * h, s_len, dp), dtype=bf16)
    v_aug[:, :, :d_head] = v_f.astype(bf16)
    v_aug[:, :, d_head] = 1.0

    in_maps = []
    for core in range(N_CORES):
        sl = slice(core * nbh, (core + 1) * nbh)
        in_maps.append(
            {
                "qT": np.ascontiguousarray(q_f[sl].transpose(0, 2, 1).astype(bf16)),
                "kT": np.ascontiguousarray(k_f[sl].transpose(0, 2, 1).astype(bf16)),
                "vA": v_aug[sl],
                "mT": mT,
            }
        )

    nc = _get_nc(nbh, s_len, d_head, scale)
    trace = bool(int(os.environ.get("KERNEL_TRACE", "0")))
    res = run_bass_kernel_spmd(nc, in_maps, list(range(N_CORES)), trace=trace)
    LAST_RESULTS = res

    out = np.empty((b * h, s_len, d_head), dtype=np.float32)
    for core in range(N_CORES):
        ot = np.asarray(res.results[core]["outT"], dtype=np.float32)
        for j in range(nbh):
            out[core * nbh + j] = (ot[j, :d_head, :] / ot[j, d_head:dp, :]).T
    return out.reshape(b, h, s_len, d_head)


# revision 21
# speedup vs baseline: 1.0152x; 1.0152x over previous
"""Masked dot-product attention, sharded over 8 NeuronCores.

Problem: B=2, H=8, S=4096, D=64; mask [1,1,S,S] broadcast over (b,h).
  scores = (Q @ K^T) / sqrt(d_k); masked_fill(~mask, -1e9); softmax;
  weights = min(mask, weights)  (== zero masked weights); out = weights @ V.

Strategy (per core: 2 of the 16 (b,h) pairs, mask replicated):
  Everything is computed in a transposed layout so no on-chip transposes
  are needed:
    s^T[k,q] = sum_d K^T[d,k] Q^T[d,q]      (PE, bf16, k-chunks of 128)
    s += 832*maskT                          (DVE scalar_tensor_tensor, in PSUM)
    w^T = exp(s/8 - 104)                    (ACT; masked lanes -> ~0)
    o^T[d,q] += V_aug^T chunks @ w^T        (PE bf16, accumulate over k; V_aug
                                             has an appended ones column so row
                                             D is the softmax denominator)
  Host side: pre-transposes Q,K (bf16) and the mask (once), appends the ones
  column to V (bf16), and does the final divide + transpose of the [D+1, S]
  fp32 output.
"""

import os
from contextlib import ExitStack

import numpy as np

B, H, S, D = 2, 8, 4096, 64
N_CORES = 8
BH_PER_CORE = (B * H) // N_CORES  # 2
KC = 128    # k-chunk size (partition dim of s^T tiles)
QB = 512    # q columns per matmul (PSUM bank limit for fp32 out)
FB = 1024   # fused free-dim for DVE/ACT elementwise ops

_BUILT = {}
LAST_RESULTS = None  # BassKernelResults of the most recent run (for test.py)


def _build(nbh, s_len, d_head, scale):
    """Build the Bass program (same SPMD program for every core)."""
    import concourse.bass as bass
    import concourse.mybir as mybir
    import concourse.tile as tile
    from concourse import bacc

    f32 = mybir.dt.float32
    bf16 = mybir.dt.bfloat16
    u8 = mybir.dt.uint8
    fb_size = min(FB, s_len)
    n_kc = s_len // KC
    n_fb = s_len // fb_size
    per_fb = fb_size // QB  # matmuls per fused elementwise block
    dp = d_head + 1

    nc = bacc.Bacc()
    qT = nc.dram_tensor("qT", [nbh, d_head, s_len], bf16, kind="ExternalInput")
    kT = nc.dram_tensor("kT", [nbh, d_head, s_len], bf16, kind="ExternalInput")
    vA = nc.dram_tensor("vA", [nbh, s_len, dp], bf16, kind="ExternalInput")
    mT = nc.dram_tensor("mT", [s_len, s_len], u8, kind="ExternalInput")
    outT = nc.dram_tensor("outT", [nbh, dp, s_len], f32, kind="ExternalOutput")

    with ExitStack() as ctx:
        tc = ctx.enter_context(tile.TileContext(nc))
        consts = ctx.enter_context(tc.tile_pool(name="consts", bufs=1))
        masks = ctx.enter_context(tc.tile_pool(name="masks", bufs=1))
        qk = ctx.enter_context(tc.tile_pool(name="qk", bufs=2))
        vpool = ctx.enter_context(tc.tile_pool(name="vpool", bufs=2))
        wpool = ctx.enter_context(tc.tile_pool(name="wpool", bufs=6))
        opool = ctx.enter_context(tc.tile_pool(name="opool", bufs=3))
        psum_s = ctx.enter_context(tc.tile_pool(name="psum_s", bufs=3, space="PSUM"))
        psum_o = ctx.enter_context(tc.tile_pool(name="psum_o", bufs=2, space="PSUM"))

        bias_sb = consts.tile([KC, 1], f32)
        nc.vector.memset(bias_sb, -104.0)

        # PE warm-up: ~7us of dummy matmuls with no DMA dependency, issued
        # while the first input DMAs are in flight. Trips the HAM clock
        # un-gate (~3.4us sustained activity) so real matmuls start at 2.4GHz.
        warm_in = consts.tile([KC, QB], bf16)
        nc.vector.memset(warm_in, 0.0)
        for wi in range(40):
            warm_ps = psum_s.tile([KC, QB], f32, tag="s_ps")
            nc.tensor.matmul(
                out=warm_ps,
                lhsT=warm_in[:, :KC],
                rhs=warm_in,
                start=True,
                stop=True,
            )

        # DMA issue order is tuned so the start-of-pipeline critical data
        # (mask chunks 0-1, bh0 q/k) lands within a few microseconds, striped
        # across many queues; the bulk of the 16 MB mask load follows.
        m_sb = masks.tile([KC, n_kc, s_len], u8)
        mT_r = mT.rearrange("(c p) q -> p c q", p=KC)
        n_mstrip = 4
        mstrip = s_len // n_mstrip

        def load_mask_chunk(c):
            for st in range(n_mstrip):
                sl = slice(st * mstrip, (st + 1) * mstrip)
                nc.sync.dma_start(out=m_sb[:, c, sl], in_=mT_r[:, c, sl])

        for c in range(2):
            load_mask_chunk(c)

        in_tiles = []
        n_strip = 8
        strip = s_len // n_strip
        for b in range(nbh):
            qt_sb = qk.tile([d_head, s_len], bf16, tag="qt")
            kt_sb = qk.tile([d_head, s_len], bf16, tag="kt")
            va_sb = vpool.tile([KC, n_kc, dp], bf16, tag="va")
            va_r = vA[b].rearrange("(c p) d -> p c d", p=KC)
            for st in range(n_strip):
                sl = slice(st * strip, (st + 1) * strip)
                nc.sync.dma_start(out=kt_sb[:, sl], in_=kT[b][:, sl])
                nc.sync.dma_start(out=qt_sb[:, sl], in_=qT[b][:, sl])
                cl = slice(st * (n_kc // n_strip), (st + 1) * (n_kc // n_strip))
                nc.sync.dma_start(out=va_sb[:, cl], in_=va_r[:, cl])
            in_tiles.append((qt_sb, kt_sb, va_sb))

        for c in range(2, n_kc):
            load_mask_chunk(c)

        for b in range(nbh):
            qt_sb, kt_sb, va_sb = in_tiles[b]

            for fb in range(n_fb):
                fs = slice(fb * fb_size, (fb + 1) * fb_size)
                o_ps = []
                for j in range(per_fb):
                    o_tile = psum_o.tile([dp, QB], f32, tag="o_ps")
                    o_ps.append(o_tile)

                for c in range(n_kc):
                    s_ps = psum_s.tile([KC, fb_size], f32, tag="s_ps")
                    for j in range(per_fb):
                        q0 = fb * fb_size + j * QB
                        nc.tensor.matmul(
                            out=s_ps[:, j * QB:(j + 1) * QB],
                            lhsT=kt_sb[:, c * KC:(c + 1) * KC],
                            rhs=qt_sb[:, q0:q0 + QB],
                            start=True,
                            stop=True,
                        )
                    # s += 832*mask  (in place in PSUM, masked lanes jump +832)
                    nc.vector.scalar_tensor_tensor(
                        out=s_ps,
                        in0=m_sb[:, c, fs],
                        scalar=832.0,
                        in1=s_ps,
                        op0=mybir.AluOpType.mult,
                        op1=mybir.AluOpType.add,
                    )
                    # w = exp(s*scale - 104): unmasked -> exp(score/8), masked -> ~0
                    w_sb = wpool.tile([KC, fb_size], bf16, tag="w")
                    nc.scalar.activation(
                        out=w_sb,
                        in_=s_ps,
                        func=mybir.ActivationFunctionType.Exp,
                        bias=bias_sb[:],
                        scale=scale,
                    )
                    for j in range(per_fb):
                        nc.tensor.matmul(
                            out=o_ps[j],
                            lhsT=va_sb[:, c, :],
                            rhs=w_sb[:, j * QB:(j + 1) * QB],
                            start=(c == 0),
                            stop=(c == n_kc - 1),
                        )
                for j in range(per_fb):
                    q0 = fb * fb_size + j * QB
                    o_sb = opool.tile([dp, QB], f32, tag="o")
                    nc.scalar.copy(o_sb, o_ps[j])
                    nc.sync.dma_start(out=outT[b, :, q0:q0 + QB], in_=o_sb)
    return nc


def _get_nc(nbh, s_len, d_head, scale):
    key = (nbh, s_len, d_head, scale)
    if key not in _BUILT:
        nc = _build(*key)
        nc.finalize()
        _BUILT[key] = nc
    return _BUILT[key]


def kernel(queries, keys, values, d_k, mask):
    global LAST_RESULTS
    import ml_dtypes
    from concourse.bass_utils import run_bass_kernel_spmd

    bf16 = ml_dtypes.bfloat16
    queries = np.asarray(queries, dtype=np.float32)
    keys = np.asarray(keys, dtype=np.float32)
    values = np.asarray(values, dtype=np.float32)
    scale = 1.0 / float(np.sqrt(np.float32(np.asarray(d_k))))

    b, h, s_len, d_head = queries.shape
    nbh = (b * h) // N_CORES
    dp = d_head + 1

    q_f = queries.reshape(b * h, s_len, d_head)
    k_f = keys.reshape(b * h, s_len, d_head)
    v_f = values.reshape(b * h, s_len, d_head)

    # mask -> uint8, transposed (shared across all cores)
    m2 = np.asarray(mask).reshape(s_len, s_len)
    mT = np.ascontiguousarray(m2.astype(np.uint8).T)

    v_aug = np.empty((b * h, s_len, dp), dtype=bf16)
    v_aug[:, :, :d_head] = v_f.astype(bf16)
    v_aug[:, :, d_head] = 1.0

    in_maps = []
    for core in range(N_CORES):
        sl = slice(core * nbh, (core + 1) * nbh)
        in_maps.append(
            {
                "qT": np.ascontiguousarray(q_f[sl].transpose(0, 2, 1).astype(bf16)),
                "kT": np.ascontiguousarray(k_f[sl].transpose(0, 2, 1).astype(bf16)),
                "vA": v_aug[sl],
                "mT": mT,
            }
        )

    nc = _get_nc(nbh, s_len, d_head, scale)
    trace = bool(int(os.environ.get("KERNEL_TRACE", "0")))
    res = run_bass_kernel_spmd(nc, in_maps, list(range(N_CORES)), trace=trace)
    LAST_RESULTS = res

    out = np.empty((b * h, s_len, d_head), dtype=np.float32)
    for core in range(N_CORES):
        ot = np.asarray(res.results[core]["outT"], dtype=np.float32)
        for j in range(nbh):
            out[core * nbh + j] = (ot[j, :d_head, :] / ot[j, d_head:dp, :]).T
    return out.reshape(b, h, s_len, d_head)


# revision 22
# speedup vs baseline: 1.0490x; 1.0333x over previous
"""Masked dot-product attention, sharded over 8 NeuronCores.

Problem: B=2, H=8, S=4096, D=64; mask [1,1,S,S] broadcast over (b,h).
  scores = (Q @ K^T) / sqrt(d_k); masked_fill(~mask, -1e9); softmax;
  weights = min(mask, weights)  (== zero masked weights); out = weights @ V.

Strategy (per core: 2 of the 16 (b,h) pairs, mask replicated):
  Everything is computed in a transposed layout so no on-chip transposes
  are needed:
    s^T[k,q] = sum_d K^T[d,k] Q^T[d,q]      (PE, bf16, k-chunks of 128)
    s += 832*maskT                          (DVE scalar_tensor_tensor, in PSUM)
    w^T = exp(s/8 - 104)                    (ACT; masked lanes -> ~0)
    o^T[d,q] += V_aug^T chunks @ w^T        (PE bf16, accumulate over k; V_aug
                                             has an appended ones column so row
                                             D is the softmax denominator)
  Host side: pre-transposes Q,K (bf16) and the mask (once), appends the ones
  column to V (bf16), and does the final divide + transpose of the [D+1, S]
  fp32 output.
"""

import os
from contextlib import ExitStack

import numpy as np

B, H, S, D = 2, 8, 4096, 64
N_CORES = 8
BH_PER_CORE = (B * H) // N_CORES  # 2
KC = 128    # k-chunk size (partition dim of s^T tiles)
QB = 512    # q columns per matmul (PSUM bank limit for fp32 out)
FB = 1024   # fused free-dim for DVE/ACT elementwise ops

_BUILT = {}
LAST_RESULTS = None  # BassKernelResults of the most recent run (for test.py)


def _build(nbh, s_len, d_head, scale):
    """Build the Bass program (same SPMD program for every core)."""
    import concourse.bass as bass
    import concourse.mybir as mybir
    import concourse.tile as tile
    from concourse import bacc

    f32 = mybir.dt.float32
    bf16 = mybir.dt.bfloat16
    u8 = mybir.dt.uint8
    fb_size = min(FB, s_len)
    n_kc = s_len // KC
    n_fb = s_len // fb_size
    per_fb = fb_size // QB  # matmuls per fused elementwise block
    dp = d_head + 1

    nc = bacc.Bacc()
    qT = nc.dram_tensor("qT", [nbh, d_head, s_len], bf16, kind="ExternalInput")
    kT = nc.dram_tensor("kT", [nbh, d_head, s_len], bf16, kind="ExternalInput")
    vA = nc.dram_tensor("vA", [nbh, s_len, dp], bf16, kind="ExternalInput")
    mT = nc.dram_tensor("mT", [s_len, s_len], u8, kind="ExternalInput")
    outT = nc.dram_tensor("outT", [nbh, dp, s_len], f32, kind="ExternalOutput")

    with ExitStack() as ctx:
        tc = ctx.enter_context(tile.TileContext(nc))
        consts = ctx.enter_context(tc.tile_pool(name="consts", bufs=1))
        masks = ctx.enter_context(tc.tile_pool(name="masks", bufs=1))
        qk = ctx.enter_context(tc.tile_pool(name="qk", bufs=2))
        vpool = ctx.enter_context(tc.tile_pool(name="vpool", bufs=2))
        wpool = ctx.enter_context(tc.tile_pool(name="wpool", bufs=6))
        opool = ctx.enter_context(tc.tile_pool(name="opool", bufs=3))
        psum_s = ctx.enter_context(tc.tile_pool(name="psum_s", bufs=3, space="PSUM"))
        psum_o = ctx.enter_context(tc.tile_pool(name="psum_o", bufs=2, space="PSUM"))

        bias_sb = consts.tile([KC, 1], f32)
        nc.vector.memset(bias_sb, -104.0)

        # PE warm-up: ~7us of dummy matmuls with no DMA dependency, issued
        # while the first input DMAs are in flight. Trips the HAM clock
        # un-gate (~3.4us sustained activity) so real matmuls start at 2.4GHz.
        warm_in = consts.tile([KC, QB], bf16)
        nc.vector.memset(warm_in, 0.0)
        for wi in range(40):
            warm_ps = psum_s.tile([KC, QB], f32, tag="s_ps")
            nc.tensor.matmul(
                out=warm_ps,
                lhsT=warm_in[:, :KC],
                rhs=warm_in,
                start=True,
                stop=True,
            )

        # DMA issue order is tuned so the start-of-pipeline critical data
        # (mask chunks 0-1, bh0 q/k) lands within a few microseconds, striped
        # across many queues; the bulk of the 16 MB mask load follows.
        m_sb = masks.tile([KC, n_kc, s_len], u8)
        mT_r = mT.rearrange("(c p) q -> p c q", p=KC)
        n_mstrip = 4
        mstrip = s_len // n_mstrip

        def load_mask_chunk(c, eng):
            for st in range(n_mstrip):
                sl = slice(st * mstrip, (st + 1) * mstrip)
                eng.dma_start(out=m_sb[:, c, sl], in_=mT_r[:, c, sl])

        # First two mask chunks striped for low latency (start of pipeline).
        load_mask_chunk(0, nc.sync)
        load_mask_chunk(1, nc.gpsimd)

        in_tiles = []
        n_strip = 4
        strip = s_len // n_strip
        for b in range(nbh):
            qt_sb = qk.tile([d_head, s_len], bf16, tag="qt")
            kt_sb = qk.tile([d_head, s_len], bf16, tag="kt")
            va_sb = vpool.tile([KC, n_kc, dp], bf16, tag="va")
            va_r = vA[b].rearrange("(c p) d -> p c d", p=KC)
            eng = nc.sync if b == 0 else nc.gpsimd
            for st in range(n_strip):
                sl = slice(st * strip, (st + 1) * strip)
                eng.dma_start(out=kt_sb[:, sl], in_=kT[b][:, sl])
                eng.dma_start(out=qt_sb[:, sl], in_=qT[b][:, sl])
                cl = slice(st * (n_kc // n_strip), (st + 1) * (n_kc // n_strip))
                eng.dma_start(out=va_sb[:, cl], in_=va_r[:, cl])
            in_tiles.append((qt_sb, kt_sb, va_sb))

        # Bulk of the mask: halved chunks, alternating issue between the two
        # DMA paths (HWDGE via sync, SWDGE via gpsimd) to engage all queues.
        for c in range(2, n_kc):
            for half in range(2):
                sl = slice(half * (s_len // 2), (half + 1) * (s_len // 2))
                eng = nc.sync if (c + half) % 2 == 0 else nc.gpsimd
                eng.dma_start(out=m_sb[:, c, sl], in_=mT_r[:, c, sl])

        for b in range(nbh):
            qt_sb, kt_sb, va_sb = in_tiles[b]

            for fb in range(n_fb):
                fs = slice(fb * fb_size, (fb + 1) * fb_size)
                o_ps = []
                for j in range(per_fb):
                    o_tile = psum_o.tile([dp, QB], f32, tag="o_ps")
                    o_ps.append(o_tile)

                for c in range(n_kc):
                    s_ps = psum_s.tile([KC, fb_size], f32, tag="s_ps")
                    for j in range(per_fb):
                        q0 = fb * fb_size + j * QB
                        nc.tensor.matmul(
                            out=s_ps[:, j * QB:(j + 1) * QB],
                            lhsT=kt_sb[:, c * KC:(c + 1) * KC],
                            rhs=qt_sb[:, q0:q0 + QB],
                            start=True,
                            stop=True,
                        )
                    # s += 832*mask  (in place in PSUM, masked lanes jump +832)
                    nc.vector.scalar_tensor_tensor(
                        out=s_ps,
                        in0=m_sb[:, c, fs],
                        scalar=832.0,
                        in1=s_ps,
                        op0=mybir.AluOpType.mult,
                        op1=mybir.AluOpType.add,
                    )
                    # w = exp(s*scale - 104): unmasked -> exp(score/8), masked -> ~0
                    w_sb = wpool.tile([KC, fb_size], bf16, tag="w")
                    nc.scalar.activation(
                        out=w_sb,
                        in_=s_ps,
                        func=mybir.ActivationFunctionType.Exp,
                        bias=bias_sb[:],
                        scale=scale,
                    )
                    for j in range(per_fb):
                        nc.tensor.matmul(
                            out=o_ps[j],
                            lhsT=va_sb[:, c, :],
                            rhs=w_sb[:, j * QB:(j + 1) * QB],
                            start=(c == 0),
                            stop=(c == n_kc - 1),
                        )
                for j in range(per_fb):
                    q0 = fb * fb_size + j * QB
                    o_sb = opool.tile([dp, QB], f32, tag="o")
                    nc.scalar.copy(o_sb, o_ps[j])
                    nc.sync.dma_start(out=outT[b, :, q0:q0 + QB], in_=o_sb)
    return nc


def _get_nc(nbh, s_len, d_head, scale):
    key = (nbh, s_len, d_head, scale)
    if key not in _BUILT:
        nc = _build(*key)
        nc.finalize()
        _BUILT[key] = nc
    return _BUILT[key]


def kernel(queries, keys, values, d_k, mask):
    global LAST_RESULTS
    import ml_dtypes
    from concourse.bass_utils import run_bass_kernel_spmd

    bf16 = ml_dtypes.bfloat16
    queries = np.asarray(queries, dtype=np.float32)
    keys = np.asarray(keys, dtype=np.float32)
    values = np.asarray(values, dtype=np.float32)
    scale = 1.0 / float(np.sqrt(np.float32(np.asarray(d_k))))

    b, h, s_len, d_head = queries.shape
    nbh = (b * h) // N_CORES
    dp = d_head + 1

    q_f = queries.reshape(b * h, s_len, d_head)
    k_f = keys.reshape(b * h, s_len, d_head)
    v_f = values.reshape(b * h, s_len, d_head)

    # mask -> uint8, transposed (shared across all cores)
    m2 = np.asarray(mask).reshape(s_len, s_len)
    mT = np.ascontiguousarray(m2.astype(np.uint8).T)

    v_aug = np.empty((b * h, s_len, dp), dtype=bf16)
    v_aug[:, :, :d_head] = v_f.astype(bf16)
    v_aug[:, :, d_head] = 1.0

    in_maps = []
    for core in range(N_CORES):
        sl = slice(core * nbh, (core + 1) * nbh)
        in_maps.append(
            {
                "qT": np.ascontiguousarray(q_f[sl].transpose(0, 2, 1).astype(bf16)),
                "kT": np.ascontiguousarray(k_f[sl].transpose(0, 2, 1).astype(bf16)),
                "vA": v_aug[sl],
                "mT": mT,
            }
        )

    nc = _get_nc(nbh, s_len, d_head, scale)
    trace = bool(int(os.environ.get("KERNEL_TRACE", "0")))
    res = run_bass_kernel_spmd(nc, in_maps, list(range(N_CORES)), trace=trace)
    LAST_RESULTS = res

    out = np.empty((b * h, s_len, d_head), dtype=np.float32)
    for core in range(N_CORES):
        ot = np.asarray(res.results[core]["outT"], dtype=np.float32)
        for j in range(nbh):
            out[core * nbh + j] = (ot[j, :d_head, :] / ot[j, d_head:dp, :]).T
    return out.reshape(b, h, s_len, d_head)
